# revision 13
# baseline (speedup 1.0000x reference)
"""CrossNet forward as a Trainium2 Bass/Tile kernel, data-parallel over 8 cores.

Math: the CrossNet layer stack
    x_{l+1} = x0 * (x_l . w_l) + b_l + x_l            (l = 0..3)
collapses in closed form.  Writing x_l = x0 * alpha_l[b] + beta_l[d]:
    p_l[b]     = sum_d x0[b,d] w_l[d]                 (4 projections of x0)
    alpha_0    = 1,   alpha_{l+1} = alpha_l * (1 + p_l) + c_l
    beta_{l+1} = beta_l + b_l,  c_l = beta_l . w_l    (host-computable scalars)
    out        = x0 * alpha_4[b] + beta_4[d]

The host rounds x to fp16 and interleaves chunk pairs so one fp32 word holds
one fp16 from each chunk of a pair.  Per 1024-row supertile the device does 4
packed fp32-dtype PE transposes (bit-exact 16-bit-halves routing), one ACT
PSUM->SBUF copy, and 8 fp16 [128d,128b]^T @ [128d,4] projection matmuls on
strided fp16 views.  Supertiles are processed in groups of 8: the projections
accumulate into one PSUM tile, ACT computes q = 1 + p on the whole group in a
single bias-add copy, and the DVE alpha recurrence runs as 2 batched ops per
group (out in fp16) instead of 3 tiny ops per supertile.  The final multiply
runs per supertile PAIR on fully contiguous fp16 views -- the output keeps the
packed (j, d, q) interleave so every tensor_tensor operand has a stride-1
16-bit last dim (DVE 2x perf mode); the host de-interleaves while upcasting.
fp16 I/O halves HBM traffic to 32 MB/core; loads issue on the SP HWDGE ring
and stores on the GpSimd SWDGE ring, one DMA per 2048-row pair.  End-to-end
error ~7e-4 (fp16 quantization of x, the projections, and alpha).
"""

import numpy as np

B = 500_000
D = 128
L = 4
N_CORES = 8
ROWS = B // N_CORES          # 62500 rows per core
G = 8                        # 128-row chunks per supertile
SUP = 128 * G                # 1024 rows per supertile
NSUP = ROWS // SUP           # 61 full supertiles
REM = ROWS - NSUP * SUP      # 36 remainder rows
GS = 4                       # supertiles per alpha group
NPAIR = G // 2               # packed chunk pairs per supertile
WPS = G * D // 2             # fp32 words per supertile per partition (512)

# Dtype for the packed pair transposes. float32's LOW_HIGH mode routes the
# two 16-bit halves bit-exactly; float32r was measured to CORRUPT packed fp16
# patterns on HW (rel err ~3.5) -- do not use it here.
TDT = "float32"

_CACHE: dict = {}

# test.py can read run metadata (exec_time_ns etc.) from here after a call.
LAST_RESULTS = None


def _build(cs, has_bias):
    import concourse.tile as tile
    from concourse import bacc, mybir

    f32 = mybir.dt.float32
    f16 = mybir.dt.float16
    tdt = getattr(mybir.dt, TDT)
    mult = mybir.AluOpType.mult
    add = mybir.AluOpType.add
    act_copy = mybir.ActivationFunctionType.Copy

    nc = bacc.Bacc(
        "TRN2",
        target_bir_lowering=False,
        debug=False,
        enable_asserts=False,
        num_devices=N_CORES,
    )
    # xp2/xp1: host-prepared fp16 supertiles, pre-grouped in PAIRS so one DMA
    # moves 2048 rows. Free layout per partition within a supertile:
    #   pair j=0..NPAIR-1 interleaved (j, d, q), chunk g = 2j+q.
    NU2 = NSUP // 2
    xp2 = nc.dram_tensor("xp2", [NU2, 128, 2 * WPS], tdt, kind="ExternalInput").ap()
    xp1 = None
    if NSUP % 2:
        xp1 = nc.dram_tensor("xp1", [128, WPS], tdt, kind="ExternalInput").ap()
    xrem = None
    if REM:
        xrem = nc.dram_tensor("xrem", [REM, D], f16, kind="ExternalInput").ap()
    w = nc.dram_tensor("w", [D, L], f16, kind="ExternalInput").ap()
    ident = nc.dram_tensor("ident", [128, 128], f16, kind="ExternalInput").ap()
    ident32 = nc.dram_tensor("ident32", [128, 128], tdt, kind="ExternalInput").ap()
    bb = bb16 = None
    if has_bias:
        bb = nc.dram_tensor("bb", [128, D], f32, kind="ExternalInput").ap()
        bb16 = nc.dram_tensor("bb16", [128, 2 * D], f16, kind="ExternalInput").ap()
    # fp16 output halves store traffic; the host upcasts to f32.  opk2/opk1
    # keep the packed (j, d, q) interleave and pair grouping; the host
    # de-interleaves.
    opk2 = nc.dram_tensor("opk2", [NU2, 128, 2 * G * D], f16, kind="ExternalOutput").ap()
    opk1 = None
    if NSUP % 2:
        opk1 = nc.dram_tensor("opk1", [128, G * D], f16, kind="ExternalOutput").ap()
    orem = None
    if REM:
        orem = nc.dram_tensor("orem", [REM, D], f16, kind="ExternalOutput").ap()

    # Supertile groups (alpha is batched per group), and DMA units of
    # adjacent supertile pairs within each group.
    groups = [list(range(a, min(a + GS, NSUP))) for a in range(0, NSUP, GS)]
    grp_of = {}
    for gi, sups in enumerate(groups):
        for s in sups:
            grp_of[s] = gi

    with tile.TileContext(nc) as tc:
        with (
            tc.tile_pool(name="consts", bufs=1) as cpool,
            tc.tile_pool(name="xin", bufs=12) as xpool,
            tc.tile_pool(name="xt", bufs=4) as xtpool,
            tc.tile_pool(name="xtps", bufs=3, space="PSUM") as tps_pool,
            tc.tile_pool(name="ptps", bufs=2, space="PSUM") as pps_pool,
            tc.tile_pool(name="small", bufs=2) as spool,
            tc.tile_pool(name="outp", bufs=6) as opool,
        ):
            ident_sb = cpool.tile([128, 128], f16, tag="ident")
            nc.sync.dma_start(ident_sb[:], ident)
            ident32_sb = cpool.tile([128, 128], tdt, tag="ident32")
            nc.sync.dma_start(ident32_sb[:], ident32)
            w_sb = cpool.tile([D, L], f16, tag="w")
            nc.sync.dma_start(w_sb[:], w)
            bb_sb = bb16_sb = None
            if has_bias:
                bb_sb = cpool.tile([128, D], f32, tag="bb")
                nc.sync.dma_start(bb_sb[:], bb)
                bb16_sb = cpool.tile([128, 2 * D], f16, tag="bb16")
                nc.sync.dma_start(bb16_sb[:], bb16)

            # Per-supertile state created by the front stage.
            xp_sb = {}   # pair-start s -> SBUF fp32 tile holding 1 or 2 supertiles
            xbase = {}   # s -> (tile, fp32 word offset of this supertile)
            xt_sb = {}   # s -> transposed fp16 chunks for the projections
            pt_t = {}    # group -> PSUM projection tile [128, 32*gsize]
            a16_t = {}   # group -> fp16 alpha tile [128, 8*gsize]

            def units_of(sups):
                us, i = [], 0
                while i < len(sups):
                    n = 2 if i + 1 < len(sups) else 1
                    us.append((sups[i], n))
                    i += n
                return us

            def front(s):
                """Load (pair units), 4 packed transposes, 1 ACT copy."""
                gi = grp_of[s]
                if gi not in pt_t:
                    gsize = len(groups[gi])
                    pt_t[gi] = pps_pool.tile(
                        [128, L * G * gsize], f32, tag="pt", name=f"pt{gi}"
                    )
                sups = groups[gi]
                first = sups[0]
                if (s - first) % 2 == 0:
                    n = 2 if s + 1 in grp_of and grp_of.get(s + 1) == gi else 1
                    if n == 2:
                        # Round-robin loads over the two HWDGE queues (2/3
                        # sync, 1/3 scalar): each DMA queue caps at ~235 GB/s,
                        # well below what the HBM sustains with several queues.
                        ring = nc.scalar if (s // 2) % 3 == 1 else nc.sync
                        t = xpool.tile([128, 2 * WPS], tdt, tag="x")
                        ring.dma_start(t[:], xp2[s // 2])
                        xp_sb[s] = t
                        xbase[s] = (t, 0)
                        xbase[s + 1] = (t, WPS)
                    else:
                        t = xpool.tile([128, WPS], tdt, tag="xs")
                        nc.sync.dma_start(t[:], xp1)
                        xp_sb[s] = t
                        xbase[s] = (t, 0)
                t, off = xbase[s]
                xt_ps = tps_pool.tile([128, WPS], tdt, tag="xtps")
                for j in range(NPAIR):
                    nc.tensor.transpose(
                        xt_ps[:, j * D : (j + 1) * D],
                        t[:, off + j * D : off + (j + 1) * D],
                        ident32_sb[:],
                    )
                xt = xtpool.tile([128, WPS], tdt, tag="xt")
                nc.scalar.copy(xt[:], xt_ps[:])
                xt_sb[s] = xt

            def mm(s):
                """8 projection matmuls into the group's PSUM tile."""
                gi = grp_of[s]
                m = s - groups[gi][0]
                xt16 = xt_sb[s][:].bitcast(f16).rearrange(
                    "d (j b q) -> d j b q", b=D, q=2
                )
                pt = pt_t[gi]
                for g in range(G):
                    j, qq = g // 2, g % 2
                    nc.tensor.matmul(
                        pt[:, (m * G + g) * L : (m * G + g + 1) * L],
                        lhsT=xt16[:, j, :, qq],
                        rhs=w_sb[:],
                        start=True,
                        stop=True,
                    )

            def alpha_group(gi):
                """q = 1 + p on ACT, then batched DVE recurrence -> fp16 alpha."""
                gsize = len(groups[gi])
                ncol = L * G * gsize
                q_sb = spool.tile([128, ncol], f32, tag="q")
                nc.scalar.activation(q_sb[:], pt_t[gi][:], act_copy, bias=1.0)
                a16 = spool.tile([128, G * gsize], f16, tag="a16")
                if has_bias:
                    qv = q_sb[:].rearrange("p (m l) -> p m l", l=L)
                    a = spool.tile([128, G * gsize], f32, tag="ah0")
                    nc.vector.tensor_copy(a[:], qv[:, :, 0])
                    for l in range(1, L):
                        tl = spool.tile([128, G * gsize], f32, tag=f"ah{l}")
                        nc.vector.tensor_mul(tl[:], a[:], qv[:, :, l])
                        if cs[l] != 0.0:
                            t2 = spool.tile([128, G * gsize], f32, tag=f"ac{l}")
                            nc.vector.tensor_scalar_add(t2[:], tl[:], float(cs[l]))
                            tl = t2
                        a = tl
                    nc.vector.tensor_copy(a16[:], a[:])
                else:
                    # alpha = (q0*q1) * (q2*q3), batched over the whole group.
                    qp = q_sb[:].rearrange("p (m u l) -> p m u l", u=2, l=2)
                    tv = spool.tile([128, 4 * G * gsize // 2], f32, tag="tv")
                    tvv = tv[:].rearrange("p (m u) -> p m u", u=2)
                    nc.vector.tensor_mul(tvv, qp[:, :, :, 0], qp[:, :, :, 1])
                    nc.vector.tensor_mul(a16[:], tvv[:, :, 0], tvv[:, :, 1])
                a16_t[gi] = a16

            def mul_store(gi):
                """Per pair: one contiguous fp16 broadcast multiply + store."""
                a16 = a16_t[gi]
                sups = groups[gi]
                for s, n in units_of(sups):
                    m = s - sups[0]
                    nj = n * NPAIR
                    t, off = xbase[s]
                    x_v = (
                        t[:, off : off + n * WPS]
                        .bitcast(f16)
                        .rearrange("p (J d q) -> p J d q", d=D, q=2)
                    )
                    a_v = (
                        a16[:, m * G : (m + n) * G]
                        .rearrange("p (J u q) -> p J u q", u=1, q=2)
                        .to_broadcast([128, nj, D, 2])
                    )
                    o_sb = opool.tile([128, n * G * D], f16, tag="o")
                    o_v = o_sb[:].rearrange("p (J d q) -> p J d q", d=D, q=2)
                    if has_bias:
                        b_v = (
                            bb16_sb[:]
                            .rearrange("p (u d q) -> p u d q", u=1, q=2)
                            .to_broadcast([128, nj, D, 2])
                        )
                        tm = opool.tile([128, n * G * D], f16, tag="t")
                        tm_v = tm[:].rearrange("p (J d q) -> p J d q", d=D, q=2)
                        nc.vector.tensor_mul(tm_v, x_v, a_v)
                        nc.vector.tensor_add(o_v, tm_v, b_v)
                    else:
                        nc.vector.tensor_mul(o_v, x_v, a_v)
                    if n == 2:
                        # Stores: 2/3 on the gpsimd SWDGE queue, 1/3 on the
                        # scalar HWDGE queue, so no single queue's ~235 GB/s
                        # cap binds either direction.
                        ring = nc.scalar if (s // 2) % 3 == 2 else nc.gpsimd
                        ring.dma_start(opk2[s // 2], o_sb[:])
                    else:
                        nc.gpsimd.dma_start(opk1, o_sb[:])

            def block_rem():
                p_cnt = REM
                x_sb = xpool.tile([p_cnt, D], f16, tag="xr")
                nc.sync.dma_start(x_sb[:], xrem)
                xt_ps = tps_pool.tile([128, p_cnt], f16, tag="xtpsr", bufs=1)
                xt = xtpool.tile([128, p_cnt], f16, tag="xtr", bufs=1)
                pt_ps = pps_pool.tile([p_cnt, L], f32, tag="ptr", bufs=1)
                nc.tensor.transpose(xt_ps[:], x_sb[:], ident_sb[:p_cnt, :p_cnt])
                nc.scalar.copy(xt[:], xt_ps[:])
                nc.tensor.matmul(
                    pt_ps[:], lhsT=xt[:], rhs=w_sb[:], start=True, stop=True
                )
                q_sb = spool.tile([p_cnt, L], f32, tag="qr")
                nc.scalar.activation(q_sb[:], pt_ps[:], act_copy, bias=1.0)
                a = spool.tile([p_cnt, 1], f32, tag="ar")
                if has_bias:
                    ah = spool.tile([p_cnt, 1], f32, tag="ahr")
                    nc.vector.tensor_copy(ah[:], q_sb[:, 0:1])
                    for l in range(1, L):
                        tl = spool.tile([p_cnt, 1], f32, tag=f"ahr{l}")
                        nc.vector.tensor_mul(tl[:], ah[:], q_sb[:, l : l + 1])
                        if cs[l] != 0.0:
                            t2 = spool.tile([p_cnt, 1], f32, tag=f"acr{l}")
                            nc.vector.tensor_scalar_add(t2[:], tl[:], float(cs[l]))
                            tl = t2
                        ah = tl
                    a = ah
                else:
                    tv = spool.tile([p_cnt, 2], f32, tag="tvr")
                    qp = q_sb[:].rearrange("p (u l) -> p u l", u=2)
                    nc.vector.tensor_mul(tv[:], qp[:, :, 0], qp[:, :, 1])
                    nc.vector.tensor_mul(a[:], tv[:, 0:1], tv[:, 1:2])
                out_sb = opool.tile([p_cnt, D], f16, tag="or")
                if has_bias:
                    nc.vector.scalar_tensor_tensor(
                        out_sb[:], x_sb[:], a[:, 0:1], bb_sb[:p_cnt, :],
                        op0=mult, op1=add,
                    )
                else:
                    nc.vector.tensor_mul(
                        out_sb[:].rearrange("p (u d) -> p u d", u=1),
                        x_sb[:].rearrange("p (u d) -> p u d", u=1),
                        a[:].to_broadcast([p_cnt, 1, D]),
                    )
                nc.gpsimd.dma_start(orem, out_sb[:])

            # Software-pipelined emission: supertile s's transposes run on PE
            # while ACT copies s-1, so the projection matmuls of s-1 (emitted
            # after front(s)) never stall PE on the copy.
            prev = None
            for s in range(NSUP):
                front(s)
                if prev is not None:
                    mm(prev)
                    if grp_of[prev] != grp_of[s]:
                        alpha_group(grp_of[prev])
                        mul_store(grp_of[prev])
                prev = s
            mm(prev)
            alpha_group(grp_of[prev])
            mul_store(grp_of[prev])
            if REM:
                block_rem()

    nc.compile()
    return nc


def _pack_shard(xs):
    # xs: [ROWS, D] float32 -> fp16 supertiles viewed as fp32 words, chunk
    # pair j interleaved (j, d, q), grouped in supertile pairs:
    #   xp2 [NSUP//2, 128, G*D] and (odd NSUP) xp1 [128, G*D/2].
    x16 = xs[: NSUP * SUP].astype(np.float16).reshape(NSUP, 128, G, D)
    pk = x16.reshape(NSUP, 128, NPAIR, 2, D)
    pk = np.ascontiguousarray(pk.transpose(0, 1, 2, 4, 3)).reshape(NSUP, 128, -1)
    n2 = NSUP // 2
    xp2 = np.ascontiguousarray(
        pk[: 2 * n2].reshape(n2, 2, 128, G * D).transpose(0, 2, 1, 3)
    ).reshape(n2, 128, 2 * G * D)
    out = {"xp2": xp2.view(np.float32)}
    if NSUP % 2:
        out["xp1"] = np.ascontiguousarray(pk[-1]).view(np.float32)
    return out


def _unpack_out(res_i):
    # opk2/opk1: packed fp16 in (j, d, q) interleave, pair-grouped ->
    # [ROWS, D] f32.
    out = np.empty((ROWS, D), dtype=np.float32)
    n2 = NSUP // 2
    pk = np.empty((NSUP, 128, G * D), dtype=np.float16)
    pk[: 2 * n2] = (
        np.asarray(res_i["opk2"])
        .reshape(n2, 128, 2, G * D)
        .transpose(0, 2, 1, 3)
        .reshape(2 * n2, 128, G * D)
    )
    if NSUP % 2:
        pk[-1] = np.asarray(res_i["opk1"])
    pk = pk.reshape(NSUP, 128, NPAIR, D, 2)
    out[: NSUP * SUP] = (
        pk.transpose(0, 1, 2, 4, 3).astype(np.float32).reshape(NSUP * SUP, D)
    )
    if REM:
        out[NSUP * SUP :] = np.asarray(res_i["orem"]).astype(np.float32)
    return out


def kernel(inputs, kernels, biases):
    global LAST_RESULTS
    import os

    if os.environ.get("BASS_TRACE"):
        # run_bass_kernel_spmd's trace path hard-imports antenv.axon_hooks,
        # which not every image ships; fall back to no-trace instead of
        # crashing when it is absent.
        try:
            import antenv.axon_hooks  # noqa: F401
        except ImportError:
            os.environ["BASS_NEVER_TRACE"] = "1"

    from concourse.bass_utils import run_bass_kernel_spmd

    x = np.ascontiguousarray(np.asarray(inputs), dtype=np.float32)
    assert x.shape == (B, D), x.shape
    kern = np.asarray(kernels, dtype=np.float32).reshape(L, D)
    bias = np.asarray(biases, dtype=np.float32).reshape(L, D)

    W = np.ascontiguousarray(kern.T)  # [D, L]
    has_bias = bool(np.any(bias))
    cs = []
    beta = np.zeros(D, dtype=np.float32)
    for l in range(L):
        cs.append(float(np.dot(beta.astype(np.float64), kern[l].astype(np.float64))))
        beta = beta + bias[l]

    key = (has_bias, tuple(cs) if has_bias else None)
    nc = _CACHE.get(key)
    if nc is None:
        nc = _build(cs, has_bias)
        _CACHE[key] = nc

    in_maps = []
    for i in range(N_CORES):
        xs = x[i * ROWS : (i + 1) * ROWS]
        m = {
            "w": W.astype(np.float16),
            "ident": np.eye(128, dtype=np.float16),
            "ident32": np.eye(128, dtype=np.float32),
        }
        m.update(_pack_shard(xs))
        if REM:
            m["xrem"] = xs[NSUP * SUP :].astype(np.float16)
        if has_bias:
            m["bb"] = np.ascontiguousarray(
                np.broadcast_to(beta, (128, D)), dtype=np.float32
            )
            b16 = np.repeat(beta.astype(np.float16), 2).reshape(1, 2 * D)
            m["bb16"] = np.ascontiguousarray(np.broadcast_to(b16, (128, 2 * D)))
        in_maps.append(m)

    res = run_bass_kernel_spmd(nc, in_maps, core_ids=list(range(N_CORES)))
    LAST_RESULTS = res
    return np.concatenate(
        [_unpack_out(res.results[i]) for i in range(N_CORES)], axis=0
    )


# revision 19
# speedup vs baseline: 1.0388x; 1.0388x over previous
"""CrossNet forward as a Trainium2 Bass/Tile kernel, data-parallel over 8 cores.

Math: the CrossNet layer stack
    x_{l+1} = x0 * (x_l . w_l) + b_l + x_l            (l = 0..3)
collapses in closed form.  Writing x_l = x0 * alpha_l[b] + beta_l[d]:
    p_l[b]     = sum_d x0[b,d] w_l[d]                 (4 projections of x0)
    alpha_0    = 1,   alpha_{l+1} = alpha_l * (1 + p_l) + c_l
    beta_{l+1} = beta_l + b_l,  c_l = beta_l . w_l    (host-computable scalars)
    out        = x0 * alpha_4[b] + beta_4[d]

The host rounds x to fp16 and interleaves chunk pairs so one fp32 word holds
one fp16 from each chunk of a pair.  Per 1024-row supertile the device does 4
packed fp32-dtype PE transposes (bit-exact 16-bit-halves routing), one ACT
PSUM->SBUF copy, and 8 fp16 [128d,128b]^T @ [128d,4] projection matmuls on
strided fp16 views.  Supertiles are processed in groups of 8: the projections
accumulate into one PSUM tile, ACT computes q = 1 + p on the whole group in a
single bias-add copy, and the DVE alpha recurrence runs as 2 batched ops per
group (out in fp16) instead of 3 tiny ops per supertile.  The final multiply
runs per supertile PAIR on fully contiguous fp16 views -- the output keeps the
packed (j, d, q) interleave so every tensor_tensor operand has a stride-1
16-bit last dim (DVE 2x perf mode); the host de-interleaves while upcasting.
fp16 I/O halves HBM traffic to 32 MB/core; loads issue on the SP HWDGE ring
and stores on the GpSimd SWDGE ring, one DMA per 2048-row pair.  End-to-end
error ~7e-4 (fp16 quantization of x, the projections, and alpha).
"""

import numpy as np

B = 500_000
D = 128
L = 4
N_CORES = 8
ROWS = B // N_CORES          # 62500 rows per core
G = 8                        # 128-row chunks per supertile
SUP = 128 * G                # 1024 rows per supertile
NSUP = ROWS // SUP           # 61 full supertiles
REM = ROWS - NSUP * SUP      # 36 remainder rows
GS = 4                       # supertiles per alpha group
NPAIR = G // 2               # packed chunk pairs per supertile
WPS = G * D // 2             # fp32 words per supertile per partition (512)

# Dtype for the packed pair transposes. float32's LOW_HIGH mode routes the
# two 16-bit halves bit-exactly; float32r was measured to CORRUPT packed fp16
# patterns on HW (rel err ~3.5) -- do not use it here.
TDT = "float32"

_CACHE: dict = {}

# test.py can read run metadata (exec_time_ns etc.) from here after a call.
LAST_RESULTS = None


def _build(cs, has_bias):
    import concourse.tile as tile
    from concourse import bacc, mybir

    f32 = mybir.dt.float32
    f16 = mybir.dt.float16
    tdt = getattr(mybir.dt, TDT)
    mult = mybir.AluOpType.mult
    add = mybir.AluOpType.add
    act_copy = mybir.ActivationFunctionType.Copy

    nc = bacc.Bacc(
        "TRN2",
        target_bir_lowering=False,
        debug=False,
        enable_asserts=False,
        num_devices=N_CORES,
    )
    # xp2/xp1: host-prepared fp16 supertiles, pre-grouped in PAIRS so one DMA
    # moves 2048 rows. Free layout per partition within a supertile:
    #   pair j=0..NPAIR-1 interleaved (j, d, q), chunk g = 2j+q.
    NU2 = NSUP // 2
    xp2 = nc.dram_tensor("xp2", [NU2, 128, 2 * WPS], tdt, kind="ExternalInput").ap()
    xp1 = None
    if NSUP % 2:
        xp1 = nc.dram_tensor("xp1", [128, WPS], tdt, kind="ExternalInput").ap()
    xrem = None
    if REM:
        xrem = nc.dram_tensor("xrem", [REM, D], f16, kind="ExternalInput").ap()
    w = nc.dram_tensor("w", [D, L], f16, kind="ExternalInput").ap()
    ident = nc.dram_tensor("ident", [128, 128], f16, kind="ExternalInput").ap()
    ident32 = nc.dram_tensor("ident32", [128, 128], tdt, kind="ExternalInput").ap()
    bb = bb16 = None
    if has_bias:
        bb = nc.dram_tensor("bb", [128, D], f32, kind="ExternalInput").ap()
        bb16 = nc.dram_tensor("bb16", [128, 2 * D], f16, kind="ExternalInput").ap()
    # fp16 output halves store traffic; the host upcasts to f32.  opk2/opk1
    # keep the packed (j, d, q) interleave and pair grouping; the host
    # de-interleaves.
    opk2 = nc.dram_tensor("opk2", [NU2, 128, 2 * G * D], f16, kind="ExternalOutput").ap()
    opk1 = None
    if NSUP % 2:
        opk1 = nc.dram_tensor("opk1", [128, G * D], f16, kind="ExternalOutput").ap()
    orem = None
    if REM:
        orem = nc.dram_tensor("orem", [REM, D], f16, kind="ExternalOutput").ap()

    # Supertile groups (alpha is batched per group), and DMA units of
    # adjacent supertile pairs within each group.
    groups = [list(range(a, min(a + GS, NSUP))) for a in range(0, NSUP, GS)]
    grp_of = {}
    for gi, sups in enumerate(groups):
        for s in sups:
            grp_of[s] = gi

    with tile.TileContext(nc) as tc:
        with (
            tc.tile_pool(name="consts", bufs=1) as cpool,
            tc.tile_pool(name="xin", bufs=18) as xpool,
            tc.tile_pool(name="xt", bufs=4) as xtpool,
            tc.tile_pool(name="xtps", bufs=3, space="PSUM") as tps_pool,
            tc.tile_pool(name="ptps", bufs=2, space="PSUM") as pps_pool,
            tc.tile_pool(name="small", bufs=2) as spool,
            tc.tile_pool(name="outp", bufs=8) as opool,
        ):
            # Consts load on the gpsimd queue (idle until the first store)
            # so the first supertile load is the sync queue's first packet.
            ident_sb = cpool.tile([128, 128], f16, tag="ident")
            nc.gpsimd.dma_start(ident_sb[:], ident)
            ident32_sb = cpool.tile([128, 128], tdt, tag="ident32")
            nc.gpsimd.dma_start(ident32_sb[:], ident32)
            w_sb = cpool.tile([D, L], f16, tag="w")
            nc.gpsimd.dma_start(w_sb[:], w)
            bb_sb = bb16_sb = None
            if has_bias:
                bb_sb = cpool.tile([128, D], f32, tag="bb")
                nc.gpsimd.dma_start(bb_sb[:], bb)
                bb16_sb = cpool.tile([128, 2 * D], f16, tag="bb16")
                nc.gpsimd.dma_start(bb16_sb[:], bb16)

            # Per-supertile state created by the front stage.
            xp_sb = {}   # pair-start s -> SBUF fp32 tile holding 1 or 2 supertiles
            xbase = {}   # s -> (tile, fp32 word offset of this supertile)
            xt_sb = {}   # s -> transposed fp16 chunks for the projections
            pt_t = {}    # group -> PSUM projection tile [128, 32*gsize]
            a16_t = {}   # group -> fp16 alpha tile [128, 8*gsize]

            def units_of(sups):
                us, i = [], 0
                while i < len(sups):
                    n = 2 if i + 1 < len(sups) else 1
                    us.append((sups[i], n))
                    i += n
                return us

            def front(s):
                """Load (pair units), 4 packed transposes, 1 ACT copy."""
                gi = grp_of[s]
                if gi not in pt_t:
                    gsize = len(groups[gi])
                    pt_t[gi] = pps_pool.tile(
                        [128, L * G * gsize], f32, tag="pt", name=f"pt{gi}"
                    )
                sups = groups[gi]
                first = sups[0]
                if (s - first) % 2 == 0:
                    n = 2 if s + 1 in grp_of and grp_of.get(s + 1) == gi else 1
                    if n == 2:
                        # Loads alternate over the two HWDGE queues (sync,
                        # scalar): each DMA queue caps at ~235 GB/s, below
                        # what HBM sustains with several queues active.
                        ring = nc.scalar if (s // 2) % 2 == 1 else nc.sync
                        t = xpool.tile([128, 2 * WPS], tdt, tag="x")
                        ring.dma_start(t[:], xp2[s // 2])
                        xp_sb[s] = t
                        xbase[s] = (t, 0)
                        xbase[s + 1] = (t, WPS)
                    else:
                        t = xpool.tile([128, WPS], tdt, tag="xs")
                        nc.sync.dma_start(t[:], xp1)
                        xp_sb[s] = t
                        xbase[s] = (t, 0)
                t, off = xbase[s]
                xt_ps = tps_pool.tile([128, WPS], tdt, tag="xtps")
                for j in range(NPAIR):
                    nc.tensor.transpose(
                        xt_ps[:, j * D : (j + 1) * D],
                        t[:, off + j * D : off + (j + 1) * D],
                        ident32_sb[:],
                    )
                xt = xtpool.tile([128, WPS], tdt, tag="xt")
                nc.scalar.copy(xt[:], xt_ps[:])
                xt_sb[s] = xt

            def mm(s):
                """8 projection matmuls into the group's PSUM tile."""
                gi = grp_of[s]
                m = s - groups[gi][0]
                xt16 = xt_sb[s][:].bitcast(f16).rearrange(
                    "d (j b q) -> d j b q", b=D, q=2
                )
                pt = pt_t[gi]
                for g in range(G):
                    j, qq = g // 2, g % 2
                    nc.tensor.matmul(
                        pt[:, (m * G + g) * L : (m * G + g + 1) * L],
                        lhsT=xt16[:, j, :, qq],
                        rhs=w_sb[:],
                        start=True,
                        stop=True,
                    )

            def alpha_group(gi):
                """q = 1 + p on ACT, then batched DVE recurrence -> fp16 alpha."""
                gsize = len(groups[gi])
                ncol = L * G * gsize
                q_sb = spool.tile([128, ncol], f32, tag="q")
                nc.scalar.activation(q_sb[:], pt_t[gi][:], act_copy, bias=1.0)
                a16 = spool.tile([128, G * gsize], f16, tag="a16")
                if has_bias:
                    qv = q_sb[:].rearrange("p (m l) -> p m l", l=L)
                    a = spool.tile([128, G * gsize], f32, tag="ah0")
                    nc.vector.tensor_copy(a[:], qv[:, :, 0])
                    for l in range(1, L):
                        tl = spool.tile([128, G * gsize], f32, tag=f"ah{l}")
                        nc.vector.tensor_mul(tl[:], a[:], qv[:, :, l])
                        if cs[l] != 0.0:
                            t2 = spool.tile([128, G * gsize], f32, tag=f"ac{l}")
                            nc.vector.tensor_scalar_add(t2[:], tl[:], float(cs[l]))
                            tl = t2
                        a = tl
                    nc.vector.tensor_copy(a16[:], a[:])
                else:
                    # alpha = (q0*q1) * (q2*q3), batched over the whole group.
                    qp = q_sb[:].rearrange("p (m u l) -> p m u l", u=2, l=2)
                    tv = spool.tile([128, 4 * G * gsize // 2], f32, tag="tv")
                    tvv = tv[:].rearrange("p (m u) -> p m u", u=2)
                    nc.vector.tensor_mul(tvv, qp[:, :, :, 0], qp[:, :, :, 1])
                    nc.vector.tensor_mul(a16[:], tvv[:, :, 0], tvv[:, :, 1])
                a16_t[gi] = a16

            def mul_store(gi):
                """Per pair: one contiguous fp16 broadcast multiply + store."""
                a16 = a16_t[gi]
                sups = groups[gi]
                for s, n in units_of(sups):
                    m = s - sups[0]
                    nj = n * NPAIR
                    t, off = xbase[s]
                    x_v = (
                        t[:, off : off + n * WPS]
                        .bitcast(f16)
                        .rearrange("p (J d q) -> p J d q", d=D, q=2)
                    )
                    a_v = (
                        a16[:, m * G : (m + n) * G]
                        .rearrange("p (J u q) -> p J u q", u=1, q=2)
                        .to_broadcast([128, nj, D, 2])
                    )
                    o_sb = opool.tile([128, n * G * D], f16, tag="o")
                    o_v = o_sb[:].rearrange("p (J d q) -> p J d q", d=D, q=2)
                    if has_bias:
                        b_v = (
                            bb16_sb[:]
                            .rearrange("p (u d q) -> p u d q", u=1, q=2)
                            .to_broadcast([128, nj, D, 2])
                        )
                        tm = opool.tile([128, n * G * D], f16, tag="t")
                        tm_v = tm[:].rearrange("p (J d q) -> p J d q", d=D, q=2)
                        nc.vector.tensor_mul(tm_v, x_v, a_v)
                        nc.vector.tensor_add(o_v, tm_v, b_v)
                    else:
                        nc.vector.tensor_mul(o_v, x_v, a_v)
                    if n == 2:
                        # Stores go on the gpsimd SWDGE queue, except the
                        # last few pairs: those ride the sync HWDGE queue,
                        # which by then has drained all its loads -- so the
                        # final store-only phase runs on two queues instead
                        # of being capped by one.
                        ring = nc.sync if s // 2 >= NU2 - 6 else nc.gpsimd
                        ring.dma_start(opk2[s // 2], o_sb[:])
                    else:
                        nc.gpsimd.dma_start(opk1, o_sb[:])

            def block_rem():
                p_cnt = REM
                x_sb = xpool.tile([p_cnt, D], f16, tag="xr")
                nc.sync.dma_start(x_sb[:], xrem)
                xt_ps = tps_pool.tile([128, p_cnt], f16, tag="xtpsr", bufs=1)
                xt = xtpool.tile([128, p_cnt], f16, tag="xtr", bufs=1)
                pt_ps = pps_pool.tile([p_cnt, L], f32, tag="ptr", bufs=1)
                nc.tensor.transpose(xt_ps[:], x_sb[:], ident_sb[:p_cnt, :p_cnt])
                nc.scalar.copy(xt[:], xt_ps[:])
                nc.tensor.matmul(
                    pt_ps[:], lhsT=xt[:], rhs=w_sb[:], start=True, stop=True
                )
                q_sb = spool.tile([p_cnt, L], f32, tag="qr")
                nc.scalar.activation(q_sb[:], pt_ps[:], act_copy, bias=1.0)
                a = spool.tile([p_cnt, 1], f32, tag="ar")
                if has_bias:
                    ah = spool.tile([p_cnt, 1], f32, tag="ahr")
                    nc.vector.tensor_copy(ah[:], q_sb[:, 0:1])
                    for l in range(1, L):
                        tl = spool.tile([p_cnt, 1], f32, tag=f"ahr{l}")
                        nc.vector.tensor_mul(tl[:], ah[:], q_sb[:, l : l + 1])
                        if cs[l] != 0.0:
                            t2 = spool.tile([p_cnt, 1], f32, tag=f"acr{l}")
                            nc.vector.tensor_scalar_add(t2[:], tl[:], float(cs[l]))
                            tl = t2
                        ah = tl
                    a = ah
                else:
                    tv = spool.tile([p_cnt, 2], f32, tag="tvr")
                    qp = q_sb[:].rearrange("p (u l) -> p u l", u=2)
                    nc.vector.tensor_mul(tv[:], qp[:, :, 0], qp[:, :, 1])
                    nc.vector.tensor_mul(a[:], tv[:, 0:1], tv[:, 1:2])
                out_sb = opool.tile([p_cnt, D], f16, tag="or")
                if has_bias:
                    nc.vector.scalar_tensor_tensor(
                        out_sb[:], x_sb[:], a[:, 0:1], bb_sb[:p_cnt, :],
                        op0=mult, op1=add,
                    )
                else:
                    nc.vector.tensor_mul(
                        out_sb[:].rearrange("p (u d) -> p u d", u=1),
                        x_sb[:].rearrange("p (u d) -> p u d", u=1),
                        a[:].to_broadcast([p_cnt, 1, D]),
                    )
                nc.gpsimd.dma_start(orem, out_sb[:])

            # Software-pipelined emission: supertile s's transposes run on PE
            # while ACT copies s-1, so the projection matmuls of s-1 (emitted
            # after front(s)) never stall PE on the copy.
            # Remainder first: its tiny load/compute/store chain hides under
            # the main stream instead of adding latency at the very end.
            if REM:
                block_rem()
            prev = None
            for s in range(NSUP):
                front(s)
                if prev is not None:
                    mm(prev)
                    if grp_of[prev] != grp_of[s]:
                        alpha_group(grp_of[prev])
                        mul_store(grp_of[prev])
                prev = s
            mm(prev)
            alpha_group(grp_of[prev])
            mul_store(grp_of[prev])

    nc.compile()
    return nc


def _pack_shard(xs):
    # xs: [ROWS, D] float32 -> fp16 supertiles viewed as fp32 words, chunk
    # pair j interleaved (j, d, q), grouped in supertile pairs:
    #   xp2 [NSUP//2, 128, G*D] and (odd NSUP) xp1 [128, G*D/2].
    x16 = xs[: NSUP * SUP].astype(np.float16).reshape(NSUP, 128, G, D)
    pk = x16.reshape(NSUP, 128, NPAIR, 2, D)
    pk = np.ascontiguousarray(pk.transpose(0, 1, 2, 4, 3)).reshape(NSUP, 128, -1)
    n2 = NSUP // 2
    xp2 = np.ascontiguousarray(
        pk[: 2 * n2].reshape(n2, 2, 128, G * D).transpose(0, 2, 1, 3)
    ).reshape(n2, 128, 2 * G * D)
    out = {"xp2": xp2.view(np.float32)}
    if NSUP % 2:
        out["xp1"] = np.ascontiguousarray(pk[-1]).view(np.float32)
    return out


def _unpack_out(res_i):
    # opk2/opk1: packed fp16 in (j, d, q) interleave, pair-grouped ->
    # [ROWS, D] f32.
    out = np.empty((ROWS, D), dtype=np.float32)
    n2 = NSUP // 2
    pk = np.empty((NSUP, 128, G * D), dtype=np.float16)
    pk[: 2 * n2] = (
        np.asarray(res_i["opk2"])
        .reshape(n2, 128, 2, G * D)
        .transpose(0, 2, 1, 3)
        .reshape(2 * n2, 128, G * D)
    )
    if NSUP % 2:
        pk[-1] = np.asarray(res_i["opk1"])
    pk = pk.reshape(NSUP, 128, NPAIR, D, 2)
    out[: NSUP * SUP] = (
        pk.transpose(0, 1, 2, 4, 3).astype(np.float32).reshape(NSUP * SUP, D)
    )
    if REM:
        out[NSUP * SUP :] = np.asarray(res_i["orem"]).astype(np.float32)
    return out


def kernel(inputs, kernels, biases):
    global LAST_RESULTS
    import os

    if os.environ.get("BASS_TRACE"):
        # run_bass_kernel_spmd's trace path hard-imports antenv.axon_hooks,
        # which not every image ships; fall back to no-trace instead of
        # crashing when it is absent.
        try:
            import antenv.axon_hooks  # noqa: F401
        except ImportError:
            os.environ["BASS_NEVER_TRACE"] = "1"

    from concourse.bass_utils import run_bass_kernel_spmd

    x = np.ascontiguousarray(np.asarray(inputs), dtype=np.float32)
    assert x.shape == (B, D), x.shape
    kern = np.asarray(kernels, dtype=np.float32).reshape(L, D)
    bias = np.asarray(biases, dtype=np.float32).reshape(L, D)

    W = np.ascontiguousarray(kern.T)  # [D, L]
    has_bias = bool(np.any(bias))
    cs = []
    beta = np.zeros(D, dtype=np.float32)
    for l in range(L):
        cs.append(float(np.dot(beta.astype(np.float64), kern[l].astype(np.float64))))
        beta = beta + bias[l]

    key = (has_bias, tuple(cs) if has_bias else None)
    nc = _CACHE.get(key)
    if nc is None:
        nc = _build(cs, has_bias)
        _CACHE[key] = nc

    in_maps = []
    for i in range(N_CORES):
        xs = x[i * ROWS : (i + 1) * ROWS]
        m = {
            "w": W.astype(np.float16),
            "ident": np.eye(128, dtype=np.float16),
            "ident32": np.eye(128, dtype=np.float32),
        }
        m.update(_pack_shard(xs))
        if REM:
            m["xrem"] = xs[NSUP * SUP :].astype(np.float16)
        if has_bias:
            m["bb"] = np.ascontiguousarray(
                np.broadcast_to(beta, (128, D)), dtype=np.float32
            )
            b16 = np.repeat(beta.astype(np.float16), 2).reshape(1, 2 * D)
            m["bb16"] = np.ascontiguousarray(np.broadcast_to(b16, (128, 2 * D)))
        in_maps.append(m)

    res = run_bass_kernel_spmd(nc, in_maps, core_ids=list(range(N_CORES)))
    LAST_RESULTS = res
    return np.concatenate(
        [_unpack_out(res.results[i]) for i in range(N_CORES)], axis=0
    )


# revision 20
# speedup vs baseline: 1.1480x; 1.1051x over previous
"""CrossNet forward as a Trainium2 Bass/Tile kernel, data-parallel over 8 cores.

Math: the CrossNet layer stack
    x_{l+1} = x0 * (x_l . w_l) + b_l + x_l            (l = 0..3)
collapses in closed form.  Writing x_l = x0 * alpha_l[b] + beta_l[d]:
    p_l[b]     = sum_d x0[b,d] w_l[d]                 (4 projections of x0)
    alpha_0    = 1,   alpha_{l+1} = alpha_l * (1 + p_l) + c_l
    beta_{l+1} = beta_l + b_l,  c_l = beta_l . w_l    (host-computable scalars)
    out        = x0 * alpha_4[b] + beta_4[d]

The host rounds x to fp16 and interleaves chunk pairs so one fp32 word holds
one fp16 from each chunk of a pair.  Per 1024-row supertile the device does 4
packed fp32-dtype PE transposes (bit-exact 16-bit-halves routing), one ACT
PSUM->SBUF copy, and 8 fp16 [128d,128b]^T @ [128d,4] projection matmuls on
strided fp16 views.  Supertiles are processed in groups of 8: the projections
accumulate into one PSUM tile, ACT computes q = 1 + p on the whole group in a
single bias-add copy, and the DVE alpha recurrence runs as 2 batched ops per
group (out in fp16) instead of 3 tiny ops per supertile.  The final multiply
runs per supertile PAIR on fully contiguous fp16 views -- the output keeps the
packed (j, d, q) interleave so every tensor_tensor operand has a stride-1
16-bit last dim (DVE 2x perf mode); the host de-interleaves while upcasting.
fp16 I/O halves HBM traffic to 32 MB/core; loads issue on the SP HWDGE ring
and stores on the GpSimd SWDGE ring, one DMA per 2048-row pair.  End-to-end
error ~7e-4 (fp16 quantization of x, the projections, and alpha).
"""

import numpy as np

B = 500_000
D = 128
L = 4
N_CORES = 8
ROWS = B // N_CORES          # 62500 rows per core
G = 8                        # 128-row chunks per supertile
SUP = 128 * G                # 1024 rows per supertile
NSUP = ROWS // SUP           # 61 full supertiles
REM = ROWS - NSUP * SUP      # 36 remainder rows
GS = 4                       # supertiles per alpha group
NPAIR = G // 2               # packed chunk pairs per supertile
WPS = G * D // 2             # fp32 words per supertile per partition (512)

# Dtype for the packed pair transposes. float32's LOW_HIGH mode routes the
# two 16-bit halves bit-exactly; float32r was measured to CORRUPT packed fp16
# patterns on HW (rel err ~3.5) -- do not use it here.
TDT = "float32"

_CACHE: dict = {}

# test.py can read run metadata (exec_time_ns etc.) from here after a call.
LAST_RESULTS = None


def _build(cs, has_bias):
    import concourse.tile as tile
    from concourse import bacc, mybir

    f32 = mybir.dt.float32
    f16 = mybir.dt.float16
    tdt = getattr(mybir.dt, TDT)
    mult = mybir.AluOpType.mult
    add = mybir.AluOpType.add
    act_copy = mybir.ActivationFunctionType.Copy

    nc = bacc.Bacc(
        "TRN2",
        target_bir_lowering=False,
        debug=False,
        enable_asserts=False,
        num_devices=N_CORES,
    )
    # xp2/xp1: host-prepared fp16 supertiles, pre-grouped in PAIRS so one DMA
    # moves 2048 rows. Free layout per partition within a supertile:
    #   pair j=0..NPAIR-1 interleaved (j, d, q), chunk g = 2j+q.
    NU2 = NSUP // 2
    xp2 = nc.dram_tensor("xp2", [NU2, 128, 2 * WPS], tdt, kind="ExternalInput").ap()
    xp1 = None
    if NSUP % 2:
        xp1 = nc.dram_tensor("xp1", [128, WPS], tdt, kind="ExternalInput").ap()
    xrem = None
    if REM:
        xrem = nc.dram_tensor("xrem", [REM, D], f16, kind="ExternalInput").ap()
    w = nc.dram_tensor("w", [D, L], f16, kind="ExternalInput").ap()
    ident = nc.dram_tensor("ident", [128, 128], f16, kind="ExternalInput").ap()
    ident32 = nc.dram_tensor("ident32", [128, 128], tdt, kind="ExternalInput").ap()
    bb = bb16 = None
    if has_bias:
        bb = nc.dram_tensor("bb", [128, D], f32, kind="ExternalInput").ap()
        bb16 = nc.dram_tensor("bb16", [128, 2 * D], f16, kind="ExternalInput").ap()
    # fp16 output halves store traffic; the host upcasts to f32.  opk2/opk1
    # keep the packed (j, d, q) interleave and pair grouping; the host
    # de-interleaves.
    opk2 = nc.dram_tensor("opk2", [NU2, 128, 2 * G * D], f16, kind="ExternalOutput").ap()
    opk1 = None
    if NSUP % 2:
        opk1 = nc.dram_tensor("opk1", [128, G * D], f16, kind="ExternalOutput").ap()
    orem = None
    if REM:
        orem = nc.dram_tensor("orem", [REM, D], f16, kind="ExternalOutput").ap()

    # Supertile groups (alpha is batched per group), and DMA units of
    # adjacent supertile pairs within each group.
    groups = [list(range(a, min(a + GS, NSUP))) for a in range(0, NSUP, GS)]
    grp_of = {}
    for gi, sups in enumerate(groups):
        for s in sups:
            grp_of[s] = gi

    with tile.TileContext(nc) as tc:
        with (
            tc.tile_pool(name="consts", bufs=1) as cpool,
            tc.tile_pool(name="xin", bufs=18) as xpool,
            tc.tile_pool(name="xt", bufs=4) as xtpool,
            tc.tile_pool(name="xtps", bufs=3, space="PSUM") as tps_pool,
            tc.tile_pool(name="ptps", bufs=2, space="PSUM") as pps_pool,
            tc.tile_pool(name="small", bufs=2) as spool,
            tc.tile_pool(name="outp", bufs=8) as opool,
        ):
            # Consts load on the gpsimd queue (idle until the first store)
            # so the first supertile load is the sync queue's first packet.
            ident_sb = cpool.tile([128, 128], f16, tag="ident")
            nc.gpsimd.dma_start(ident_sb[:], ident)
            ident32_sb = cpool.tile([128, 128], tdt, tag="ident32")
            nc.gpsimd.dma_start(ident32_sb[:], ident32)
            w_sb = cpool.tile([D, L], f16, tag="w")
            nc.gpsimd.dma_start(w_sb[:], w)
            bb_sb = bb16_sb = None
            if has_bias:
                bb_sb = cpool.tile([128, D], f32, tag="bb")
                nc.gpsimd.dma_start(bb_sb[:], bb)
                bb16_sb = cpool.tile([128, 2 * D], f16, tag="bb16")
                nc.gpsimd.dma_start(bb16_sb[:], bb16)

            # Per-supertile state created by the front stage.
            xp_sb = {}   # pair-start s -> SBUF fp32 tile holding 1 or 2 supertiles
            xbase = {}   # s -> (tile, fp32 word offset of this supertile)
            xt_sb = {}   # s -> transposed fp16 chunks for the projections
            pt_t = {}    # group -> PSUM projection tile [128, 32*gsize]
            a16_t = {}   # group -> fp16 alpha tile [128, 8*gsize]

            def units_of(sups):
                us, i = [], 0
                while i < len(sups):
                    n = 2 if i + 1 < len(sups) else 1
                    us.append((sups[i], n))
                    i += n
                return us

            def front(s):
                """Load (pair units), 4 packed transposes, 1 ACT copy."""
                gi = grp_of[s]
                if gi not in pt_t:
                    gsize = len(groups[gi])
                    pt_t[gi] = pps_pool.tile(
                        [128, L * G * gsize], f32, tag="pt", name=f"pt{gi}"
                    )
                sups = groups[gi]
                first = sups[0]
                if (s - first) % 2 == 0:
                    n = 2 if s + 1 in grp_of and grp_of.get(s + 1) == gi else 1
                    if n == 2:
                        # All loads on the sync HWDGE queue: splitting onto
                        # the scalar queue was measured slower (~182 GB/s
                        # there, and total DMA is fabric-capped at ~358 GB/s
                        # anyway).
                        t = xpool.tile([128, 2 * WPS], tdt, tag="x")
                        nc.sync.dma_start(t[:], xp2[s // 2])
                        xp_sb[s] = t
                        xbase[s] = (t, 0)
                        xbase[s + 1] = (t, WPS)
                    else:
                        t = xpool.tile([128, WPS], tdt, tag="xs")
                        nc.sync.dma_start(t[:], xp1)
                        xp_sb[s] = t
                        xbase[s] = (t, 0)
                t, off = xbase[s]
                xt_ps = tps_pool.tile([128, WPS], tdt, tag="xtps")
                for j in range(NPAIR):
                    nc.tensor.transpose(
                        xt_ps[:, j * D : (j + 1) * D],
                        t[:, off + j * D : off + (j + 1) * D],
                        ident32_sb[:],
                    )
                xt = xtpool.tile([128, WPS], tdt, tag="xt")
                nc.scalar.copy(xt[:], xt_ps[:])
                xt_sb[s] = xt

            def mm(s):
                """8 projection matmuls into the group's PSUM tile."""
                gi = grp_of[s]
                m = s - groups[gi][0]
                xt16 = xt_sb[s][:].bitcast(f16).rearrange(
                    "d (j b q) -> d j b q", b=D, q=2
                )
                pt = pt_t[gi]
                for g in range(G):
                    j, qq = g // 2, g % 2
                    nc.tensor.matmul(
                        pt[:, (m * G + g) * L : (m * G + g + 1) * L],
                        lhsT=xt16[:, j, :, qq],
                        rhs=w_sb[:],
                        start=True,
                        stop=True,
                    )

            def alpha_group(gi):
                """q = 1 + p on ACT, then batched DVE recurrence -> fp16 alpha."""
                gsize = len(groups[gi])
                ncol = L * G * gsize
                q_sb = spool.tile([128, ncol], f32, tag="q")
                nc.scalar.activation(q_sb[:], pt_t[gi][:], act_copy, bias=1.0)
                a16 = spool.tile([128, G * gsize], f16, tag="a16")
                if has_bias:
                    qv = q_sb[:].rearrange("p (m l) -> p m l", l=L)
                    a = spool.tile([128, G * gsize], f32, tag="ah0")
                    nc.vector.tensor_copy(a[:], qv[:, :, 0])
                    for l in range(1, L):
                        tl = spool.tile([128, G * gsize], f32, tag=f"ah{l}")
                        nc.vector.tensor_mul(tl[:], a[:], qv[:, :, l])
                        if cs[l] != 0.0:
                            t2 = spool.tile([128, G * gsize], f32, tag=f"ac{l}")
                            nc.vector.tensor_scalar_add(t2[:], tl[:], float(cs[l]))
                            tl = t2
                        a = tl
                    nc.vector.tensor_copy(a16[:], a[:])
                else:
                    # alpha = (q0*q1) * (q2*q3), batched over the whole group.
                    qp = q_sb[:].rearrange("p (m u l) -> p m u l", u=2, l=2)
                    tv = spool.tile([128, 4 * G * gsize // 2], f32, tag="tv")
                    tvv = tv[:].rearrange("p (m u) -> p m u", u=2)
                    nc.vector.tensor_mul(tvv, qp[:, :, :, 0], qp[:, :, :, 1])
                    nc.vector.tensor_mul(a16[:], tvv[:, :, 0], tvv[:, :, 1])
                a16_t[gi] = a16

            def mul_store(gi):
                """Per pair: one contiguous fp16 broadcast multiply + store."""
                a16 = a16_t[gi]
                sups = groups[gi]
                for s, n in units_of(sups):
                    m = s - sups[0]
                    nj = n * NPAIR
                    t, off = xbase[s]
                    x_v = (
                        t[:, off : off + n * WPS]
                        .bitcast(f16)
                        .rearrange("p (J d q) -> p J d q", d=D, q=2)
                    )
                    a_v = (
                        a16[:, m * G : (m + n) * G]
                        .rearrange("p (J u q) -> p J u q", u=1, q=2)
                        .to_broadcast([128, nj, D, 2])
                    )
                    o_sb = opool.tile([128, n * G * D], f16, tag="o")
                    o_v = o_sb[:].rearrange("p (J d q) -> p J d q", d=D, q=2)
                    if has_bias:
                        b_v = (
                            bb16_sb[:]
                            .rearrange("p (u d q) -> p u d q", u=1, q=2)
                            .to_broadcast([128, nj, D, 2])
                        )
                        tm = opool.tile([128, n * G * D], f16, tag="t")
                        tm_v = tm[:].rearrange("p (J d q) -> p J d q", d=D, q=2)
                        nc.vector.tensor_mul(tm_v, x_v, a_v)
                        nc.vector.tensor_add(o_v, tm_v, b_v)
                    else:
                        nc.vector.tensor_mul(o_v, x_v, a_v)
                    if n == 2:
                        # Stores go on the gpsimd SWDGE queue, except the
                        # last few pairs: those ride the sync HWDGE queue,
                        # which by then has drained all its loads -- so the
                        # final store-only phase runs on two queues instead
                        # of being capped by one.
                        ring = nc.sync if s // 2 >= NU2 - 6 else nc.gpsimd
                        ring.dma_start(opk2[s // 2], o_sb[:])
                    else:
                        nc.gpsimd.dma_start(opk1, o_sb[:])

            def block_rem():
                p_cnt = REM
                x_sb = xpool.tile([p_cnt, D], f16, tag="xr")
                nc.sync.dma_start(x_sb[:], xrem)
                xt_ps = tps_pool.tile([128, p_cnt], f16, tag="xtpsr", bufs=1)
                xt = xtpool.tile([128, p_cnt], f16, tag="xtr", bufs=1)
                pt_ps = pps_pool.tile([p_cnt, L], f32, tag="ptr", bufs=1)
                nc.tensor.transpose(xt_ps[:], x_sb[:], ident_sb[:p_cnt, :p_cnt])
                nc.scalar.copy(xt[:], xt_ps[:])
                nc.tensor.matmul(
                    pt_ps[:], lhsT=xt[:], rhs=w_sb[:], start=True, stop=True
                )
                q_sb = spool.tile([p_cnt, L], f32, tag="qr")
                nc.scalar.activation(q_sb[:], pt_ps[:], act_copy, bias=1.0)
                a = spool.tile([p_cnt, 1], f32, tag="ar")
                if has_bias:
                    ah = spool.tile([p_cnt, 1], f32, tag="ahr")
                    nc.vector.tensor_copy(ah[:], q_sb[:, 0:1])
                    for l in range(1, L):
                        tl = spool.tile([p_cnt, 1], f32, tag=f"ahr{l}")
                        nc.vector.tensor_mul(tl[:], ah[:], q_sb[:, l : l + 1])
                        if cs[l] != 0.0:
                            t2 = spool.tile([p_cnt, 1], f32, tag=f"acr{l}")
                            nc.vector.tensor_scalar_add(t2[:], tl[:], float(cs[l]))
                            tl = t2
                        ah = tl
                    a = ah
                else:
                    tv = spool.tile([p_cnt, 2], f32, tag="tvr")
                    qp = q_sb[:].rearrange("p (u l) -> p u l", u=2)
                    nc.vector.tensor_mul(tv[:], qp[:, :, 0], qp[:, :, 1])
                    nc.vector.tensor_mul(a[:], tv[:, 0:1], tv[:, 1:2])
                out_sb = opool.tile([p_cnt, D], f16, tag="or")
                if has_bias:
                    nc.vector.scalar_tensor_tensor(
                        out_sb[:], x_sb[:], a[:, 0:1], bb_sb[:p_cnt, :],
                        op0=mult, op1=add,
                    )
                else:
                    nc.vector.tensor_mul(
                        out_sb[:].rearrange("p (u d) -> p u d", u=1),
                        x_sb[:].rearrange("p (u d) -> p u d", u=1),
                        a[:].to_broadcast([p_cnt, 1, D]),
                    )
                nc.gpsimd.dma_start(orem, out_sb[:])

            # Software-pipelined emission: supertile s's transposes run on PE
            # while ACT copies s-1, so the projection matmuls of s-1 (emitted
            # after front(s)) never stall PE on the copy.
            # Remainder first: its tiny load/compute/store chain hides under
            # the main stream instead of adding latency at the very end.
            if REM:
                block_rem()
            prev = None
            for s in range(NSUP):
                front(s)
                if prev is not None:
                    mm(prev)
                    if grp_of[prev] != grp_of[s]:
                        alpha_group(grp_of[prev])
                        mul_store(grp_of[prev])
                prev = s
            mm(prev)
            alpha_group(grp_of[prev])
            mul_store(grp_of[prev])

    nc.compile()
    return nc


def _pack_shard(xs):
    # xs: [ROWS, D] float32 -> fp16 supertiles viewed as fp32 words, chunk
    # pair j interleaved (j, d, q), grouped in supertile pairs:
    #   xp2 [NSUP//2, 128, G*D] and (odd NSUP) xp1 [128, G*D/2].
    x16 = xs[: NSUP * SUP].astype(np.float16).reshape(NSUP, 128, G, D)
    pk = x16.reshape(NSUP, 128, NPAIR, 2, D)
    pk = np.ascontiguousarray(pk.transpose(0, 1, 2, 4, 3)).reshape(NSUP, 128, -1)
    n2 = NSUP // 2
    xp2 = np.ascontiguousarray(
        pk[: 2 * n2].reshape(n2, 2, 128, G * D).transpose(0, 2, 1, 3)
    ).reshape(n2, 128, 2 * G * D)
    out = {"xp2": xp2.view(np.float32)}
    if NSUP % 2:
        out["xp1"] = np.ascontiguousarray(pk[-1]).view(np.float32)
    return out


def _unpack_out(res_i):
    # opk2/opk1: packed fp16 in (j, d, q) interleave, pair-grouped ->
    # [ROWS, D] f32.
    out = np.empty((ROWS, D), dtype=np.float32)
    n2 = NSUP // 2
    pk = np.empty((NSUP, 128, G * D), dtype=np.float16)
    pk[: 2 * n2] = (
        np.asarray(res_i["opk2"])
        .reshape(n2, 128, 2, G * D)
        .transpose(0, 2, 1, 3)
        .reshape(2 * n2, 128, G * D)
    )
    if NSUP % 2:
        pk[-1] = np.asarray(res_i["opk1"])
    pk = pk.reshape(NSUP, 128, NPAIR, D, 2)
    out[: NSUP * SUP] = (
        pk.transpose(0, 1, 2, 4, 3).astype(np.float32).reshape(NSUP * SUP, D)
    )
    if REM:
        out[NSUP * SUP :] = np.asarray(res_i["orem"]).astype(np.float32)
    return out


def kernel(inputs, kernels, biases):
    global LAST_RESULTS
    import os

    if os.environ.get("BASS_TRACE"):
        # run_bass_kernel_spmd's trace path hard-imports antenv.axon_hooks,
        # which not every image ships; fall back to no-trace instead of
        # crashing when it is absent.
        try:
            import antenv.axon_hooks  # noqa: F401
        except ImportError:
            os.environ["BASS_NEVER_TRACE"] = "1"

    from concourse.bass_utils import run_bass_kernel_spmd

    x = np.ascontiguousarray(np.asarray(inputs), dtype=np.float32)
    assert x.shape == (B, D), x.shape
    kern = np.asarray(kernels, dtype=np.float32).reshape(L, D)
    bias = np.asarray(biases, dtype=np.float32).reshape(L, D)

    W = np.ascontiguousarray(kern.T)  # [D, L]
    has_bias = bool(np.any(bias))
    cs = []
    beta = np.zeros(D, dtype=np.float32)
    for l in range(L):
        cs.append(float(np.dot(beta.astype(np.float64), kern[l].astype(np.float64))))
        beta = beta + bias[l]

    key = (has_bias, tuple(cs) if has_bias else None)
    nc = _CACHE.get(key)
    if nc is None:
        nc = _build(cs, has_bias)
        _CACHE[key] = nc

    in_maps = []
    for i in range(N_CORES):
        xs = x[i * ROWS : (i + 1) * ROWS]
        m = {
            "w": W.astype(np.float16),
            "ident": np.eye(128, dtype=np.float16),
            "ident32": np.eye(128, dtype=np.float32),
        }
        m.update(_pack_shard(xs))
        if REM:
            m["xrem"] = xs[NSUP * SUP :].astype(np.float16)
        if has_bias:
            m["bb"] = np.ascontiguousarray(
                np.broadcast_to(beta, (128, D)), dtype=np.float32
            )
            b16 = np.repeat(beta.astype(np.float16), 2).reshape(1, 2 * D)
            m["bb16"] = np.ascontiguousarray(np.broadcast_to(b16, (128, 2 * D)))
        in_maps.append(m)

    res = run_bass_kernel_spmd(nc, in_maps, core_ids=list(range(N_CORES)))
    LAST_RESULTS = res
    return np.concatenate(
        [_unpack_out(res.results[i]) for i in range(N_CORES)], axis=0
    )
